# revision 7
# baseline (speedup 1.0000x reference)
"""Trainium2 Bass kernel for DerivativeNet (per-pixel 3-tap derivative stencils).

Computation (per batch b, C=1):
  out_x = nmask * (xK0*u[w-1] + xK1*u[w] + xK2*u[w+1])   (zero-padded in W)
  out_y = nmask * (yK0*u[h-1] + yK1*u[h] + yK2*u[h+1])   (zero-padded in H)
  output = stack([out_x, out_y])  -> [2, B, 1, H, W]

Sharding: pure data parallel over B=8 across the 8 NeuronCores (one batch
element per core). Per-core HBM traffic is ~40MB (memory-bound regime).

Implementation notes:
- u is zero-padded to [H+2, W+2] on the host, so every stencil edge case is
  an ordinary in-bounds read; the kernel needs no memsets (memsets run on a
  separate sequencer proc and burn a sync-wait slot on their consumers -
  walrus codegen allows only 2 waits per compute instruction).
- Compute-engine APs must start at partition 0/32/64/96, so the h-stencil
  row shifts are SBUF->SBUF DMA copies (DMA writes at any partition):
    u_t[p]    = u_pad[r0+p]    "up" tap reads it directly
    u_c_t[p]  = u_pad[r0+1+p]  center rows (full padded width for w-stencil)
    u_dn_t[p] = u_pad[r0+2+p]  "down" tap rows
- 12 elementwise fp32 ops per tile, split 8 on VectorE / 4 on GpSimd (fp32
  tensor_tensor runs 1x on DVE and never contends with GpSimd SBUF ports).
- A 1-element "sync absorber" copy per iteration makes the DVE engine
  observe GpSimd's progress once, so the scratch-slot WAR releases don't
  add a 3rd sync-wait to the tap instructions.
- Loads ride the SP HWDGE ring; shifts + stores ride the ACT HWDGE ring.
"""

import numpy as np

import concourse.bass as bass
import concourse.bacc as bacc
import concourse.mybir as mybir
from concourse.tile import TileContext
from concourse.bass_utils import run_bass_kernel_spmd

H = 1024
W = 1024
B = 8
N_CORES = 8
ROWS = 126  # output rows per tile iteration (u tile holds n+2 rows -> <=128 partitions)
F32 = mybir.dt.float32

LAST_RESULTS = None  # test.py reads profiling info from here


def _build() -> bass.Bass:
    # Bacc (not plain Bass): its compile() runs generate_event_semaphores,
    # which splits multi-sem waits into separate instructions (TRN2 allows
    # at most one embedded sync wait per compute instruction).
    nc = bacc.Bacc("TRN2", target_bir_lowering=False)
    u_d = nc.dram_tensor("u", [H + 2, W + 2], F32, kind="ExternalInput")
    nm_d = nc.dram_tensor("nmask", [H, W], F32, kind="ExternalInput")
    xk_d = nc.dram_tensor("xK", [3, H, W], F32, kind="ExternalInput")
    yk_d = nc.dram_tensor("yK", [3, H, W], F32, kind="ExternalInput")
    out_d = nc.dram_tensor("out", [2, H, W], F32, kind="ExternalOutput")

    mult = mybir.AluOpType.mult
    add = mybir.AluOpType.add

    with TileContext(nc) as tc:
        with (
            tc.tile_pool(name="io", bufs=2) as io,
            tc.tile_pool(name="sc", bufs=2) as sc,
            tc.tile_pool(name="mini", bufs=1) as mini,
        ):
            prev_out_t = None
            r0 = 0
            while r0 < H:
                n = min(ROWS, H - r0)

                # u_pad rows r0 .. r0+n+1 at partitions 0..n+1 (padded width)
                u_t = io.tile([128, W + 2], F32, name="u_t", tag="u_t")
                nc.sync.dma_start(out=u_t[0 : n + 2, :], in_=u_d[r0 : r0 + n + 2, :])

                if prev_out_t is not None:
                    # sync absorber: one DVE read of the previous iteration's
                    # GpSimd output advances DVE's observed GpSimd clock, so
                    # the taps below need no slot-release waits of their own.
                    dummy = mini.tile([1, 1], F32, name="dummy", tag="dummy")
                    nc.vector.tensor_copy(dummy[0:1, :], prev_out_t[0:1, 0, 0:1])

                # row-shifted copies (partition-offset reads are legal for DMA)
                u_c_t = io.tile([128, W + 2], F32, name="u_c_t", tag="u_c_t")
                nc.scalar.dma_start(out=u_c_t[0:n, :], in_=u_t[1 : n + 1, :])
                u_dn_t = io.tile([128, W], F32, name="u_dn_t", tag="u_dn_t")
                nc.scalar.dma_start(out=u_dn_t[0:n, :], in_=u_t[2 : n + 2, 1 : W + 1])

                nm_t = io.tile([128, W], F32, name="nm_t", tag="nm_t")
                nc.sync.dma_start(out=nm_t[0:n, :], in_=nm_d[r0 : r0 + n, :])
                xk_t = io.tile([128, 3, W], F32, name="xk_t", tag="xk_t")
                nc.sync.dma_start(
                    out=xk_t[0:n], in_=xk_d[:, r0 : r0 + n, :].rearrange("t h w -> h t w")
                )
                yk_t = io.tile([128, 3, W], F32, name="yk_t", tag="yk_t")
                nc.sync.dma_start(
                    out=yk_t[0:n], in_=yk_d[:, r0 : r0 + n, :].rearrange("t h w -> h t w")
                )

                out_t = io.tile([128, 2, W], F32, name="out_t", tag="out_t")

                ax = sc.tile([128, W], F32, name="ax", tag="ax")
                bx = sc.tile([128, W], F32, name="bx", tag="bx")
                cx = sc.tile([128, W], F32, name="cx", tag="cx")
                ay = sc.tile([128, W], F32, name="ay", tag="ay")
                by = sc.tile([128, W], F32, name="by", tag="by")
                cy = sc.tile([128, W], F32, name="cy", tag="cy")

                # taps (VectorE)
                nc.vector.tensor_tensor(ax[0:n], xk_t[0:n, 0, :], u_c_t[0:n, 0:W], mult)
                nc.vector.tensor_tensor(bx[0:n], xk_t[0:n, 2, :], u_c_t[0:n, 2 : W + 2], mult)
                nc.vector.tensor_tensor(cx[0:n], xk_t[0:n, 1, :], u_c_t[0:n, 1 : W + 1], mult)
                nc.vector.tensor_tensor(ay[0:n], yk_t[0:n, 0, :], u_t[0:n, 1 : W + 1], mult)
                nc.vector.tensor_tensor(by[0:n], yk_t[0:n, 2, :], u_dn_t[0:n, :], mult)
                nc.vector.tensor_tensor(cy[0:n], yk_t[0:n, 1, :], u_c_t[0:n, 1 : W + 1], mult)
                # first partial sums (VectorE, in place)
                nc.vector.tensor_tensor(ax[0:n], ax[0:n], bx[0:n], add)
                nc.vector.tensor_tensor(ay[0:n], ay[0:n], by[0:n], add)
                # second sums + mask multiplies (GpSimd)
                nc.gpsimd.tensor_tensor(cx[0:n], cx[0:n], ax[0:n], add)
                nc.gpsimd.tensor_tensor(cy[0:n], cy[0:n], ay[0:n], add)
                nc.gpsimd.tensor_tensor(out_t[0:n, 0, :], cx[0:n], nm_t[0:n], mult)
                nc.gpsimd.tensor_tensor(out_t[0:n, 1, :], cy[0:n], nm_t[0:n], mult)

                # store both planes in one DMA on the ACT HWDGE ring
                nc.scalar.dma_start(
                    out=out_d[:, r0 : r0 + n, :].rearrange("c h w -> h c w"),
                    in_=out_t[0:n],
                )
                prev_out_t = out_t
                r0 += n
    nc.compile()
    return nc


_PROGRAM = None


def _get_program() -> bass.Bass:
    global _PROGRAM
    if _PROGRAM is None:
        _PROGRAM = _build()
    return _PROGRAM


def kernel(u, nmask, xK, yK):
    global LAST_RESULTS
    nc = _get_program()

    u = np.asarray(u, dtype=np.float32)
    nmask = np.asarray(nmask, dtype=np.float32)
    xK = np.asarray(xK, dtype=np.float32)
    yK = np.asarray(yK, dtype=np.float32)

    in_maps = []
    for b in range(B):
        u_pad = np.zeros((H + 2, W + 2), dtype=np.float32)
        u_pad[1 : H + 1, 1 : W + 1] = u[b, 0]
        in_maps.append(
            {
                "u": u_pad,
                "nmask": np.ascontiguousarray(nmask[b, 0]),
                "xK": np.ascontiguousarray(xK[b, 0, 0]),  # [3, H, W]
                "yK": np.ascontiguousarray(yK[b, 0, :, 0]),  # [3, H, W]
            }
        )

    res = run_bass_kernel_spmd(nc, in_maps, core_ids=list(range(N_CORES)))
    LAST_RESULTS = res

    outs = [r["out"] for r in res.results]  # each [2, H, W]
    full = np.stack(outs, axis=1)  # [2, B, H, W]
    return full[:, :, None, :, :].astype(np.float32)  # [2, B, 1, H, W]


# revision 8
# speedup vs baseline: 1.2216x; 1.2216x over previous
"""Trainium2 Bass kernel for DerivativeNet (per-pixel 3-tap derivative stencils).

Computation (per batch b, C=1):
  out_x = nmask * (xK0*u[w-1] + xK1*u[w] + xK2*u[w+1])   (zero-padded in W)
  out_y = nmask * (yK0*u[h-1] + yK1*u[h] + yK2*u[h+1])   (zero-padded in H)
  output = stack([out_x, out_y])  -> [2, B, 1, H, W]

Sharding: pure data parallel over B=8 across the 8 NeuronCores (one batch
element per core). Per-core HBM traffic is ~42MB (memory-bound regime).

Implementation notes:
- u is zero-padded to [H+2, W+2] on the host, so every stencil edge case is
  an ordinary in-bounds read.
- Compute-engine APs must start at partition 0/32/64/96, so the h-stencil
  row shifts are done on the otherwise-idle TensorEngine: multiply u_t by a
  constant shifted-identity matrix (embedded in the NEFF via inline_tensor),
  producing the shift-by-1 (center) and shift-by-2 (down) row copies in
  PSUM. The "up" tap reads u_t directly at partition offset 0. This keeps
  all row-shift traffic off the DMA rings (v1 used SBUF->SBUF DMA copies,
  which added ~20% DMA traffic and a serial load->shift dependency).
- 12 elementwise fp32 ops per tile, split 8 on VectorE / 4 on GpSimd (fp32
  tensor_tensor runs 1x on DVE and never contends with GpSimd SBUF ports).
  VectorE reads the shifted rows straight from PSUM (fp32 TT is 1x-rate for
  PSUM operands too); GpSimd ops touch only SBUF (it has no PSUM port).
- The w-stencil edge columns are handled by narrowing the two outer-tap ops
  by one column and zeroing the edge column of their outputs.
- A 1-element "sync absorber" copy per iteration makes the DVE engine
  observe GpSimd's progress once per iteration, minimizing per-instruction
  sync waits (Bacc.compile splits >1-wait instructions, but fewer is faster).
- Loads are split across both HWDGE rings (SP: u+xk+nmask, ACT: yk+store)
  to balance descriptor generation.
"""

import numpy as np

import concourse.bass as bass
import concourse.bacc as bacc
import concourse.mybir as mybir
from concourse.tile import TileContext
from concourse.bass_utils import run_bass_kernel_spmd

H = 1024
W = 1024
B = 8
N_CORES = 8
ROWS = 126  # output rows per tile iteration (u tile holds n+2 rows -> <=128 partitions)
F32 = mybir.dt.float32

LAST_RESULTS = None  # test.py reads profiling info from here


def _build() -> bass.Bass:
    # Bacc (not plain Bass): its compile() runs generate_event_semaphores,
    # which splits multi-sem waits into separate instructions (TRN2 allows
    # at most one embedded sync wait per compute instruction).
    nc = bacc.Bacc("TRN2", target_bir_lowering=False)
    u_d = nc.dram_tensor("u", [H + 2, W + 2], F32, kind="ExternalInput")
    nm_d = nc.dram_tensor("nmask", [H, W], F32, kind="ExternalInput")
    xk_d = nc.dram_tensor("xK", [3, H, W], F32, kind="ExternalInput")
    yk_d = nc.dram_tensor("yK", [3, H, W], F32, kind="ExternalInput")
    out_d = nc.dram_tensor("out", [2, H, W], F32, kind="ExternalOutput")

    # shifted identity matrices: S1[k, p] = [k == p+1], S2[k, p] = [k == p+2]
    # (lhsT layout: out[p, :] = sum_k S[k, p] * rhs[k, :] = rhs[p+shift, :])
    sdata = np.zeros((128, 256), dtype=np.float32)
    for p in range(127):
        sdata[p + 1, p] = 1.0
    for p in range(126):
        sdata[p + 2, 128 + p] = 1.0
    shift_d = nc.inline_tensor(sdata, name="shiftmat")

    mult = mybir.AluOpType.mult
    add = mybir.AluOpType.add

    with TileContext(nc) as tc:
        with (
            tc.tile_pool(name="io", bufs=3) as io,
            tc.tile_pool(name="sc", bufs=2) as sc,
            tc.tile_pool(name="ps", bufs=2, space="PSUM") as ps,
            tc.tile_pool(name="mini", bufs=1) as mini,
        ):
            s_t = mini.tile([128, 256], F32, name="s_t", tag="s_t")
            nc.sync.dma_start(out=s_t[:, :], in_=shift_d[:, :])

            prev_out_t = None
            r0 = 0
            while r0 < H:
                n = min(ROWS, H - r0)
                k = n + 2  # rows of u_pad held on chip / matmul contraction dim

                # u_pad rows r0 .. r0+n+1 at partitions 0..n+1 (padded width)
                u_t = io.tile([128, W + 2], F32, name="u_t", tag="u_t")
                nc.sync.dma_start(out=u_t[0:k, :], in_=u_d[r0 : r0 + k, :])

                if prev_out_t is not None:
                    # sync absorber: one DVE read of the previous iteration's
                    # GpSimd output advances DVE's observed GpSimd clock, so
                    # the scratch-slot releases below need no waits of their own.
                    dummy = mini.tile([1, 1], F32, name="dummy", tag="dummy")
                    nc.vector.tensor_copy(dummy[0:1, :], prev_out_t[0:1, 0, 0:1])

                # row-shifted copies via TensorE: uc[p] = u_pad[r0+1+p],
                # udn[p] = u_pad[r0+2+p], both over true u columns 0..W-1.
                uc_ps = ps.tile([128, W], F32, name="uc_ps", tag="uc_ps")
                udn_ps = ps.tile([128, W], F32, name="udn_ps", tag="udn_ps")
                for j in (0, 512):
                    nc.tensor.matmul(
                        uc_ps[:, j : j + 512],
                        s_t[0:k, 0:128],
                        u_t[0:k, 1 + j : 513 + j],
                        start=True,
                        stop=True,
                    )
                    nc.tensor.matmul(
                        udn_ps[:, j : j + 512],
                        s_t[0:k, 128:256],
                        u_t[0:k, 1 + j : 513 + j],
                        start=True,
                        stop=True,
                    )

                nm_t = io.tile([128, W], F32, name="nm_t", tag="nm_t")
                nc.sync.dma_start(out=nm_t[0:n, :], in_=nm_d[r0 : r0 + n, :])
                xk_t = io.tile([128, 3, W], F32, name="xk_t", tag="xk_t")
                nc.sync.dma_start(
                    out=xk_t[0:n], in_=xk_d[:, r0 : r0 + n, :].rearrange("t h w -> h t w")
                )
                yk_t = io.tile([128, 3, W], F32, name="yk_t", tag="yk_t")
                nc.scalar.dma_start(
                    out=yk_t[0:n], in_=yk_d[:, r0 : r0 + n, :].rearrange("t h w -> h t w")
                )

                out_t = io.tile([128, 2, W], F32, name="out_t", tag="out_t")

                ax = sc.tile([128, W], F32, name="ax", tag="ax")
                bx = sc.tile([128, W], F32, name="bx", tag="bx")
                cx = sc.tile([128, W], F32, name="cx", tag="cx")
                ay = sc.tile([128, W], F32, name="ay", tag="ay")
                by = sc.tile([128, W], F32, name="by", tag="by")
                cy = sc.tile([128, W], F32, name="cy", tag="cy")

                # taps (VectorE; uc/udn operands live in PSUM)
                # out_x left tap: u[w-1] -> col 0 output is the W zero-pad edge
                nc.vector.tensor_tensor(
                    ax[0:n, 1:W], xk_t[0:n, 0, 1:W], uc_ps[0:n, 0 : W - 1], mult
                )
                nc.vector.memset(ax[0:n, 0:1], 0.0)
                # out_x right tap: u[w+1] -> col W-1 output is the zero-pad edge
                nc.vector.tensor_tensor(
                    bx[0:n, 0 : W - 1], xk_t[0:n, 2, 0 : W - 1], uc_ps[0:n, 1:W], mult
                )
                nc.vector.memset(bx[0:n, W - 1 : W], 0.0)
                nc.vector.tensor_tensor(cx[0:n], xk_t[0:n, 1, :], uc_ps[0:n, :], mult)
                nc.vector.tensor_tensor(ay[0:n], yk_t[0:n, 0, :], u_t[0:n, 1 : W + 1], mult)
                nc.vector.tensor_tensor(by[0:n], yk_t[0:n, 2, :], udn_ps[0:n, :], mult)
                nc.vector.tensor_tensor(cy[0:n], yk_t[0:n, 1, :], uc_ps[0:n, :], mult)
                # first partial sums (VectorE, in place)
                nc.vector.tensor_tensor(ax[0:n], ax[0:n], bx[0:n], add)
                nc.vector.tensor_tensor(ay[0:n], ay[0:n], by[0:n], add)
                # second sums + mask multiplies (GpSimd; SBUF operands only)
                nc.gpsimd.tensor_tensor(cx[0:n], cx[0:n], ax[0:n], add)
                nc.gpsimd.tensor_tensor(cy[0:n], cy[0:n], ay[0:n], add)
                nc.gpsimd.tensor_tensor(out_t[0:n, 0, :], cx[0:n], nm_t[0:n], mult)
                nc.gpsimd.tensor_tensor(out_t[0:n, 1, :], cy[0:n], nm_t[0:n], mult)

                # store both planes in one DMA on the ACT HWDGE ring
                nc.scalar.dma_start(
                    out=out_d[:, r0 : r0 + n, :].rearrange("c h w -> h c w"),
                    in_=out_t[0:n],
                )
                prev_out_t = out_t
                r0 += n
    nc.compile()
    return nc


_PROGRAM = None


def _get_program() -> bass.Bass:
    global _PROGRAM
    if _PROGRAM is None:
        _PROGRAM = _build()
    return _PROGRAM


def kernel(u, nmask, xK, yK):
    global LAST_RESULTS
    nc = _get_program()

    u = np.asarray(u, dtype=np.float32)
    nmask = np.asarray(nmask, dtype=np.float32)
    xK = np.asarray(xK, dtype=np.float32)
    yK = np.asarray(yK, dtype=np.float32)

    in_maps = []
    for b in range(B):
        u_pad = np.zeros((H + 2, W + 2), dtype=np.float32)
        u_pad[1 : H + 1, 1 : W + 1] = u[b, 0]
        in_maps.append(
            {
                "u": u_pad,
                "nmask": np.ascontiguousarray(nmask[b, 0]),
                "xK": np.ascontiguousarray(xK[b, 0, 0]),  # [3, H, W]
                "yK": np.ascontiguousarray(yK[b, 0, :, 0]),  # [3, H, W]
            }
        )

    res = run_bass_kernel_spmd(nc, in_maps, core_ids=list(range(N_CORES)))
    LAST_RESULTS = res

    outs = [r["out"] for r in res.results]  # each [2, H, W]
    full = np.stack(outs, axis=1)  # [2, B, H, W]
    return full[:, :, None, :, :].astype(np.float32)  # [2, B, 1, H, W]


# revision 12
# speedup vs baseline: 1.2655x; 1.0359x over previous
"""Trainium2 Bass kernel for DerivativeNet (per-pixel 3-tap derivative stencils).

Computation (per batch b, C=1):
  out_x = nmask * (xK0*u[w-1] + xK1*u[w] + xK2*u[w+1])   (zero-padded in W)
  out_y = nmask * (yK0*u[h-1] + yK1*u[h] + yK2*u[h+1])   (zero-padded in H)
  output = stack([out_x, out_y])  -> [2, B, 1, H, W]

Sharding: pure data parallel over B=8 across the 8 NeuronCores (one batch
element per core). Per-core HBM traffic is ~42MB (memory-bound regime).

Implementation notes:
- u is zero-padded to [H+2, W+2] on the host, so every stencil edge case is
  an ordinary in-bounds read.
- Compute-engine APs must start at partition 0/32/64/96, so the h-stencil
  row shifts are done on the otherwise-idle TensorEngine: multiply u_t by a
  constant shifted-identity matrix (embedded in the NEFF via inline_tensor),
  producing the shift-by-1 (center) and shift-by-2 (down) row copies in
  PSUM. The "up" tap reads u_t directly at partition offset 0. This keeps
  all row-shift traffic off the DMA rings (v1 used SBUF->SBUF DMA copies,
  which added ~20% DMA traffic and a serial load->shift dependency).
- 12 elementwise fp32 ops per tile, split 8 on VectorE / 4 on GpSimd (fp32
  tensor_tensor runs 1x on DVE and never contends with GpSimd SBUF ports).
  VectorE reads the shifted rows straight from PSUM (fp32 TT is 1x-rate for
  PSUM operands too); GpSimd ops touch only SBUF (it has no PSUM port).
- The w-stencil edge columns are handled by narrowing the two outer-tap ops
  by one column and zeroing the edge column of their outputs.
- A 1-element "sync absorber" copy per iteration makes the DVE engine
  observe GpSimd's progress once per iteration, minimizing per-instruction
  sync waits (Bacc.compile splits >1-wait instructions, but fewer is faster).
- Loads are split across both HWDGE rings (SP: u+xk+nmask, ACT: yk+store)
  to balance descriptor generation.
"""

import numpy as np

import concourse.bass as bass
import concourse.bacc as bacc
import concourse.mybir as mybir
from concourse.tile import TileContext
from concourse.bass_utils import run_bass_kernel_spmd

H = 1024
W = 1024
B = 8
N_CORES = 8
ROWS = 126  # output rows per tile iteration (u tile holds n+2 rows -> <=128 partitions)
F32 = mybir.dt.float32

LAST_RESULTS = None  # test.py reads profiling info from here


def _build() -> bass.Bass:
    # Bacc (not plain Bass): its compile() runs generate_event_semaphores,
    # which splits multi-sem waits into separate instructions (TRN2 allows
    # at most one embedded sync wait per compute instruction).
    nc = bacc.Bacc("TRN2", target_bir_lowering=False)
    u_d = nc.dram_tensor("u", [H + 2, W + 2], F32, kind="ExternalInput")
    nm_d = nc.dram_tensor("nmask", [H, W], F32, kind="ExternalInput")
    xk_d = nc.dram_tensor("xK", [3, H, W], F32, kind="ExternalInput")
    yk_d = nc.dram_tensor("yK", [3, H, W], F32, kind="ExternalInput")
    out_d = nc.dram_tensor("out", [2, H, W], F32, kind="ExternalOutput")

    # shifted identity matrices: S1[k, p] = [k == p+1], S2[k, p] = [k == p+2]
    # (lhsT layout: out[p, :] = sum_k S[k, p] * rhs[k, :] = rhs[p+shift, :])
    sdata = np.zeros((128, 256), dtype=np.float32)
    for p in range(127):
        sdata[p + 1, p] = 1.0
    for p in range(126):
        sdata[p + 2, 128 + p] = 1.0
    shift_d = nc.inline_tensor(sdata, name="shiftmat")

    mult = mybir.AluOpType.mult
    add = mybir.AluOpType.add

    with TileContext(nc) as tc:
        with (
            tc.tile_pool(name="io", bufs=3) as io,
            tc.tile_pool(name="sc", bufs=2) as sc,
            tc.tile_pool(name="ps", bufs=2, space="PSUM") as ps,
            tc.tile_pool(name="mini", bufs=1) as mini,
        ):
            s_t = mini.tile([128, 256], F32, name="s_t", tag="s_t")
            nc.sync.dma_start(out=s_t[:, :], in_=shift_d[:, :])

            prev_out_t = None
            r0 = 0
            while r0 < H:
                n = min(ROWS, H - r0)
                k = n + 2  # rows of u_pad held on chip / matmul contraction dim

                # u_pad rows r0 .. r0+n+1 at partitions 0..n+1 (padded width)
                u_t = io.tile([128, W + 2], F32, name="u_t", tag="u_t", bufs=4)
                nc.sync.dma_start(out=u_t[0:k, :], in_=u_d[r0 : r0 + k, :])

                if prev_out_t is not None:
                    # sync absorber: one DVE read of the previous iteration's
                    # GpSimd output advances DVE's observed GpSimd clock, so
                    # the scratch-slot releases below need no waits of their own.
                    dummy = mini.tile([1, 1], F32, name="dummy", tag="dummy")
                    nc.vector.tensor_copy(dummy[0:1, :], prev_out_t[0:1, 0, 0:1])

                # row-shifted copies via TensorE: uc[p] = u_pad[r0+1+p],
                # udn[p] = u_pad[r0+2+p], both over true u columns 0..W-1.
                # (fp32 matmul is the exact 9-pass path; float32r would need
                # pre-rounded inputs and loses mantissa bits)
                uc_ps = ps.tile([128, W], F32, name="uc_ps", tag="uc_ps")
                udn_ps = ps.tile([128, W], F32, name="udn_ps", tag="udn_ps")
                for j in (0, 512):
                    nc.tensor.matmul(
                        uc_ps[:, j : j + 512],
                        s_t[0:k, 0:128],
                        u_t[0:k, 1 + j : 513 + j],
                        start=True,
                        stop=True,
                    )
                    nc.tensor.matmul(
                        udn_ps[:, j : j + 512],
                        s_t[0:k, 128:256],
                        u_t[0:k, 1 + j : 513 + j],
                        start=True,
                        stop=True,
                    )

                nm_t = io.tile([128, W], F32, name="nm_t", tag="nm_t")
                nc.sync.dma_start(out=nm_t[0:n, :], in_=nm_d[r0 : r0 + n, :])
                xk_t = io.tile([128, 3, W], F32, name="xk_t", tag="xk_t")
                nc.sync.dma_start(
                    out=xk_t[0:n], in_=xk_d[:, r0 : r0 + n, :].rearrange("t h w -> h t w")
                )
                yk_t = io.tile([128, 3, W], F32, name="yk_t", tag="yk_t")
                nc.scalar.dma_start(
                    out=yk_t[0:n], in_=yk_d[:, r0 : r0 + n, :].rearrange("t h w -> h t w")
                )

                out_t = io.tile([128, 2, W], F32, name="out_t", tag="out_t")

                ax = sc.tile([128, W], F32, name="ax", tag="ax")
                bx = sc.tile([128, W], F32, name="bx", tag="bx")
                cx = sc.tile([128, W], F32, name="cx", tag="cx")
                ay = sc.tile([128, W], F32, name="ay", tag="ay")
                by = sc.tile([128, W], F32, name="by", tag="by")
                cy = sc.tile([128, W], F32, name="cy", tag="cy")

                # taps (VectorE; uc/udn operands live in PSUM)
                # out_x left tap: u[w-1] -> col 0 output is the W zero-pad edge
                nc.vector.tensor_tensor(
                    ax[0:n, 1:W], xk_t[0:n, 0, 1:W], uc_ps[0:n, 0 : W - 1], mult
                )
                nc.vector.memset(ax[0:n, 0:1], 0.0)
                # out_x right tap: u[w+1] -> col W-1 output is the zero-pad edge
                nc.vector.tensor_tensor(
                    bx[0:n, 0 : W - 1], xk_t[0:n, 2, 0 : W - 1], uc_ps[0:n, 1:W], mult
                )
                nc.vector.memset(bx[0:n, W - 1 : W], 0.0)
                nc.vector.tensor_tensor(cx[0:n], xk_t[0:n, 1, :], uc_ps[0:n, :], mult)
                nc.vector.tensor_tensor(ay[0:n], yk_t[0:n, 0, :], u_t[0:n, 1 : W + 1], mult)
                nc.vector.tensor_tensor(by[0:n], yk_t[0:n, 2, :], udn_ps[0:n, :], mult)
                nc.vector.tensor_tensor(cy[0:n], yk_t[0:n, 1, :], uc_ps[0:n, :], mult)
                # first partial sums (VectorE, in place)
                nc.vector.tensor_tensor(ax[0:n], ax[0:n], bx[0:n], add)
                nc.vector.tensor_tensor(ay[0:n], ay[0:n], by[0:n], add)
                # second sums + mask multiplies (GpSimd; SBUF operands only),
                # with each output plane stored as soon as it is ready
                nc.gpsimd.tensor_tensor(cx[0:n], cx[0:n], ax[0:n], add)
                nc.gpsimd.tensor_tensor(out_t[0:n, 0, :], cx[0:n], nm_t[0:n], mult)
                nc.scalar.dma_start(out=out_d[0, r0 : r0 + n, :], in_=out_t[0:n, 0, :])
                nc.gpsimd.tensor_tensor(cy[0:n], cy[0:n], ay[0:n], add)
                nc.gpsimd.tensor_tensor(out_t[0:n, 1, :], cy[0:n], nm_t[0:n], mult)
                nc.scalar.dma_start(out=out_d[1, r0 : r0 + n, :], in_=out_t[0:n, 1, :])
                prev_out_t = out_t
                r0 += n
    nc.compile()
    return nc


_PROGRAM = None


def _get_program() -> bass.Bass:
    global _PROGRAM
    if _PROGRAM is None:
        _PROGRAM = _build()
    return _PROGRAM


def kernel(u, nmask, xK, yK):
    global LAST_RESULTS
    nc = _get_program()

    u = np.asarray(u, dtype=np.float32)
    nmask = np.asarray(nmask, dtype=np.float32)
    xK = np.asarray(xK, dtype=np.float32)
    yK = np.asarray(yK, dtype=np.float32)

    in_maps = []
    for b in range(B):
        u_pad = np.zeros((H + 2, W + 2), dtype=np.float32)
        u_pad[1 : H + 1, 1 : W + 1] = u[b, 0]
        in_maps.append(
            {
                "u": u_pad,
                "nmask": np.ascontiguousarray(nmask[b, 0]),
                "xK": np.ascontiguousarray(xK[b, 0, 0]),  # [3, H, W]
                "yK": np.ascontiguousarray(yK[b, 0, :, 0]),  # [3, H, W]
            }
        )

    res = run_bass_kernel_spmd(nc, in_maps, core_ids=list(range(N_CORES)))
    LAST_RESULTS = res

    outs = [r["out"] for r in res.results]  # each [2, H, W]
    full = np.stack(outs, axis=1)  # [2, B, H, W]
    return full[:, :, None, :, :].astype(np.float32)  # [2, B, 1, H, W]


# revision 15
# speedup vs baseline: 1.3894x; 1.0979x over previous
"""Trainium2 Bass kernel for DerivativeNet (per-pixel 3-tap derivative stencils).

Computation (per batch b, C=1):
  out_x = nmask * (xK0*u[w-1] + xK1*u[w] + xK2*u[w+1])   (zero-padded in W)
  out_y = nmask * (yK0*u[h-1] + yK1*u[h] + yK2*u[h+1])   (zero-padded in H)
  output = stack([out_x, out_y])  -> [2, B, 1, H, W]

Sharding: pure data parallel over B=8 across the 8 NeuronCores (one batch
element per core). Per-core HBM traffic is ~42MB (memory-bound regime).

Implementation notes:
- u is zero-padded to [H+2, W+2] on the host, so every stencil edge case is
  an ordinary in-bounds read.
- Compute-engine APs must start at partition 0/32/64/96, so the h-stencil
  row shifts are done on the otherwise-idle TensorEngine: multiply u_t by a
  constant shifted-identity matrix (embedded in the NEFF via inline_tensor),
  producing the shift-by-1 (center) and shift-by-2 (down) row copies in
  PSUM. The "up" tap reads u_t directly at partition offset 0. This keeps
  all row-shift traffic off the DMA rings (v1 used SBUF->SBUF DMA copies,
  which added ~20% DMA traffic and a serial load->shift dependency).
- 12 elementwise fp32 ops per tile, split 8 on VectorE / 4 on GpSimd (fp32
  tensor_tensor runs 1x on DVE and never contends with GpSimd SBUF ports).
  VectorE reads the shifted rows straight from PSUM (fp32 TT is 1x-rate for
  PSUM operands too); GpSimd ops touch only SBUF (it has no PSUM port).
- The w-stencil edge columns are handled by narrowing the two outer-tap ops
  by one column and zeroing the edge column of their outputs.
- A 1-element "sync absorber" copy per iteration makes the DVE engine
  observe GpSimd's progress once per iteration, minimizing per-instruction
  sync waits (Bacc.compile splits >1-wait instructions, but fewer is faster).
- Loads are split across both HWDGE rings (SP: u+xk+nmask, ACT: yk+store)
  to balance descriptor generation.
"""

import numpy as np

import concourse.bass as bass
import concourse.bacc as bacc
import concourse.mybir as mybir
from concourse.tile import TileContext
from concourse.bass_utils import run_bass_kernel_spmd

H = 1024
W = 1024
B = 8
N_CORES = 8
ROWS = 126  # output rows per tile iteration (u tile holds n+2 rows -> <=128 partitions)
F32 = mybir.dt.float32

LAST_RESULTS = None  # test.py reads profiling info from here


def _build() -> bass.Bass:
    # Bacc (not plain Bass): its compile() runs generate_event_semaphores,
    # which splits multi-sem waits into separate instructions (TRN2 allows
    # at most one embedded sync wait per compute instruction).
    nc = bacc.Bacc("TRN2", target_bir_lowering=False)
    u_d = nc.dram_tensor("u", [H + 2, W + 2], F32, kind="ExternalInput")
    nm_d = nc.dram_tensor("nmask", [H, W], F32, kind="ExternalInput")
    xk_d = nc.dram_tensor("xK", [3, H, W], F32, kind="ExternalInput")
    yk_d = nc.dram_tensor("yK", [3, H, W], F32, kind="ExternalInput")
    out_d = nc.dram_tensor("out", [2, H, W], F32, kind="ExternalOutput")

    # shifted identity matrices: S1[k, p] = [k == p+1], S2[k, p] = [k == p+2]
    # (lhsT layout: out[p, :] = sum_k S[k, p] * rhs[k, :] = rhs[p+shift, :])
    sdata = np.zeros((128, 256), dtype=np.float32)
    for p in range(127):
        sdata[p + 1, p] = 1.0
    for p in range(126):
        sdata[p + 2, 128 + p] = 1.0
    shift_d = nc.inline_tensor(sdata, name="shiftmat")

    mult = mybir.AluOpType.mult
    add = mybir.AluOpType.add

    with TileContext(nc) as tc:
        with (
            tc.tile_pool(name="io", bufs=3) as io,
            tc.tile_pool(name="sc", bufs=2) as sc,
            tc.tile_pool(name="ps", bufs=2, space="PSUM") as ps,
            tc.tile_pool(name="mini", bufs=1) as mini,
        ):
            s_t = mini.tile([128, 256], F32, name="s_t", tag="s_t")
            nc.sync.dma_start(out=s_t[:, :], in_=shift_d[:, :])

            out_t_hist = []  # per-iteration out_t handles for the sync absorber
            r0 = 0
            while r0 < H:
                n = min(ROWS, H - r0)
                k = n + 2  # rows of u_pad held on chip / matmul contraction dim

                # u_pad rows r0 .. r0+n+1 at partitions 0..n+1 (padded width)
                u_t = io.tile([128, W + 2], F32, name="u_t", tag="u_t", bufs=4)
                nc.sync.dma_start(out=u_t[0:k, :], in_=u_d[r0 : r0 + k, :])

                if len(out_t_hist) >= 2:
                    # sync absorber: one DVE read of the i-2 iteration's GpSimd
                    # output advances DVE's observed GpSimd clock far enough to
                    # cover the scratch-slot releases (bufs=2 -> the slots being
                    # reused were last read by GpSimd in iteration i-2), without
                    # serializing DVE behind the i-1 GpSimd work.
                    dummy = mini.tile([1, 1], F32, name="dummy", tag="dummy")
                    nc.vector.tensor_copy(dummy[0:1, :], out_t_hist[-2][0:1, 0, 0:1])

                # row-shifted copies via TensorE: uc[p] = u_pad[r0+1+p],
                # udn[p] = u_pad[r0+2+p], both over true u columns 0..W-1.
                # (fp32 matmul is the exact 9-pass path; float32r would need
                # pre-rounded inputs and loses mantissa bits)
                uc_ps = ps.tile([128, W], F32, name="uc_ps", tag="uc_ps")
                udn_ps = ps.tile([128, W], F32, name="udn_ps", tag="udn_ps")
                # group by stationary matrix so ldweights can be reused
                for sl, dst in ((0, uc_ps), (128, udn_ps)):
                    for j in (0, 512):
                        nc.tensor.matmul(
                            dst[:, j : j + 512],
                            s_t[0:k, sl : sl + 128],
                            u_t[0:k, 1 + j : 513 + j],
                            start=True,
                            stop=True,
                        )

                nm_t = io.tile([128, W], F32, name="nm_t", tag="nm_t")
                nc.sync.dma_start(out=nm_t[0:n, :], in_=nm_d[r0 : r0 + n, :])
                xk_t = io.tile([128, 3, W], F32, name="xk_t", tag="xk_t")
                nc.sync.dma_start(
                    out=xk_t[0:n], in_=xk_d[:, r0 : r0 + n, :].rearrange("t h w -> h t w")
                )
                yk_t = io.tile([128, 3, W], F32, name="yk_t", tag="yk_t")
                nc.scalar.dma_start(
                    out=yk_t[0:n], in_=yk_d[:, r0 : r0 + n, :].rearrange("t h w -> h t w")
                )

                out_t = io.tile([128, 2, W], F32, name="out_t", tag="out_t")

                ax = sc.tile([128, W], F32, name="ax", tag="ax")
                bx = sc.tile([128, W], F32, name="bx", tag="bx")
                cx = sc.tile([128, W], F32, name="cx", tag="cx")
                ay = sc.tile([128, W], F32, name="ay", tag="ay")
                by = sc.tile([128, W], F32, name="by", tag="by")
                cy = sc.tile([128, W], F32, name="cy", tag="cy")

                # taps (VectorE; uc/udn operands live in PSUM)
                # out_x left tap: u[w-1] -> col 0 output is the W zero-pad edge
                nc.vector.tensor_tensor(
                    ax[0:n, 1:W], xk_t[0:n, 0, 1:W], uc_ps[0:n, 0 : W - 1], mult
                )
                nc.vector.memset(ax[0:n, 0:1], 0.0)
                # out_x right tap: u[w+1] -> col W-1 output is the zero-pad edge
                nc.vector.tensor_tensor(
                    bx[0:n, 0 : W - 1], xk_t[0:n, 2, 0 : W - 1], uc_ps[0:n, 1:W], mult
                )
                nc.vector.memset(bx[0:n, W - 1 : W], 0.0)
                nc.vector.tensor_tensor(cx[0:n], xk_t[0:n, 1, :], uc_ps[0:n, :], mult)
                nc.vector.tensor_tensor(ay[0:n], yk_t[0:n, 0, :], u_t[0:n, 1 : W + 1], mult)
                nc.vector.tensor_tensor(by[0:n], yk_t[0:n, 2, :], udn_ps[0:n, :], mult)
                nc.vector.tensor_tensor(cy[0:n], yk_t[0:n, 1, :], uc_ps[0:n, :], mult)
                # first partial sums (VectorE, in place)
                nc.vector.tensor_tensor(ax[0:n], ax[0:n], bx[0:n], add)
                nc.vector.tensor_tensor(ay[0:n], ay[0:n], by[0:n], add)
                # second sums + mask multiplies (GpSimd; SBUF operands only),
                # with each output plane stored as soon as it is ready
                nc.gpsimd.tensor_tensor(cx[0:n], cx[0:n], ax[0:n], add)
                nc.gpsimd.tensor_tensor(out_t[0:n, 0, :], cx[0:n], nm_t[0:n], mult)
                nc.scalar.dma_start(out=out_d[0, r0 : r0 + n, :], in_=out_t[0:n, 0, :])
                nc.gpsimd.tensor_tensor(cy[0:n], cy[0:n], ay[0:n], add)
                nc.gpsimd.tensor_tensor(out_t[0:n, 1, :], cy[0:n], nm_t[0:n], mult)
                nc.scalar.dma_start(out=out_d[1, r0 : r0 + n, :], in_=out_t[0:n, 1, :])
                out_t_hist.append(out_t)
                r0 += n
    nc.compile()
    return nc


_PROGRAM = None


def _get_program() -> bass.Bass:
    global _PROGRAM
    if _PROGRAM is None:
        _PROGRAM = _build()
    return _PROGRAM


def kernel(u, nmask, xK, yK):
    global LAST_RESULTS
    nc = _get_program()

    u = np.asarray(u, dtype=np.float32)
    nmask = np.asarray(nmask, dtype=np.float32)
    xK = np.asarray(xK, dtype=np.float32)
    yK = np.asarray(yK, dtype=np.float32)

    in_maps = []
    for b in range(B):
        u_pad = np.zeros((H + 2, W + 2), dtype=np.float32)
        u_pad[1 : H + 1, 1 : W + 1] = u[b, 0]
        in_maps.append(
            {
                "u": u_pad,
                "nmask": np.ascontiguousarray(nmask[b, 0]),
                "xK": np.ascontiguousarray(xK[b, 0, 0]),  # [3, H, W]
                "yK": np.ascontiguousarray(yK[b, 0, :, 0]),  # [3, H, W]
            }
        )

    res = run_bass_kernel_spmd(nc, in_maps, core_ids=list(range(N_CORES)))
    LAST_RESULTS = res

    outs = [r["out"] for r in res.results]  # each [2, H, W]
    full = np.stack(outs, axis=1)  # [2, B, H, W]
    return full[:, :, None, :, :].astype(np.float32)  # [2, B, 1, H, W]
